# revision 1
# baseline (speedup 1.0000x reference)
"""2-layer GCN (SpMM message passing) on 8 Trainium2 NeuronCores — v2.

Strategy (dest-row-sharded SpMM, replicated dense layer-1 matmul):
  - Dest nodes are relabeled to (core, block, slot): 8 cores x BLOCKS blocks
    x 64 slots.  Per-core bin packing (snake by degree + repair) bounds each
    block's in-edge count by CPB*128.
  - Every core computes the FULL Z = feat @ W1 (replicating this cheap
    matmul beats AllGather-ing Z shards) and writes it bf16 to its DRAM.
  - Per core, edges are grouped by dest block and sorted by source col;
    the 128-edge chunks of a super-block (8 blocks) are interleaved by col
    band and packed into dma_gather calls of up to MAXG chunks whose
    cross-core col span fits the int16 window (32767 rows).
  - Aggregation: per chunk j, matmul lhsT=gathered[128 edges, feat-half],
    rhs=S cols (one-hot(slot)*w built on DVE per call) accumulating into
    the chunk's block slice of a [128, 512] PSUM bank per feat half.
  - Per super-block: relu+bias on the whole bank, Y = H @ W2 per 128-row
    pair, then AllGather Y and layer 2 runs the same schedule on yall.
"""

import numpy as np

P = 128
RB = 64            # rows (slots) per block
F_IN = 256
F_HID = 256
F_OUT = 128
WIN = 32767        # dma_gather int16 index window
NQ = 4             # SWDGE queues for gathers (ucode max)
import os as _os_mod
FP8_FEAT = _os_mod.environ.get("GCN_FP8_FEAT", "1") == "1"
FP8_Z = _os_mod.environ.get("GCN_FP8_Z", "1") == "1"
SG = 32            # chunks per S-matrix generation group (amortizes DVE ops)


class Cfg:
    def __init__(self, n_nodes, n_edges, ncores, cpb, maxg=48, sbb=8):
        assert n_nodes % ncores == 0
        self.n = n_nodes
        self.e = n_edges
        self.ncores = ncores
        self.npc = n_nodes // ncores
        nb = (self.npc + RB - 1) // RB
        if nb * RB - self.npc < 32:      # slack rows for bin packing
            nb += 1
        if nb % sbb:
            nb += sbb - nb % sbb         # whole super-blocks
        self.blocks = nb
        self.sbb = sbb                   # blocks per super-block
        self.nsb = nb // sbb
        self.cpb = cpb                   # chunks (of 128 edges) per block
        self.maxg = maxg                 # max chunks per dma_gather call
        self.totch = nb * cpb
        self.bp = nb * RB                # padded nodes per core
        self.ntot = ncores * self.bp


def full_cfg():
    # per-block capacity cpb*128=2048 vs ~2000 mean edges; bin packing keeps
    # every block under cap (assert-checked).
    import os
    maxg = int(os.environ.get("GCN_MAXG", "8"))  # >8 overflows the Q7 idx scratch and hangs
    return Cfg(100000, 3200000, 8, cpb=16, maxg=maxg, sbb=8)


# --------------------------------------------------------------------------
# Host-side preprocessing
# --------------------------------------------------------------------------

def _assign_nodes(cfg, deg_in):
    """node -> (core, block, slot), balancing per-core and per-block edges."""
    n, ncores, nb = cfg.n, cfg.ncores, cfg.blocks
    order = np.argsort(-deg_in, kind="stable")
    pos = np.arange(n)
    phase = pos % (2 * ncores)
    core_of_pos = np.where(phase < ncores, phase, 2 * ncores - 1 - phase)
    node_core = np.empty(n, dtype=np.int64)
    node_core[order] = core_of_pos

    node_block = np.empty(n, dtype=np.int64)
    node_slot = np.empty(n, dtype=np.int64)
    cap_edges = cfg.cpb * P
    for c in range(ncores):
        nodes = order[core_of_pos == c]          # degree-desc within core
        m = len(nodes)
        assert m == cfg.npc
        bpos = np.arange(m)
        ph = bpos % (2 * nb)
        blk = np.where(ph < nb, ph, 2 * nb - 1 - ph)
        cnt = np.bincount(blk, minlength=nb)
        esum = np.bincount(blk, weights=deg_in[nodes], minlength=nb)
        assert cnt.max() <= RB, f"block row overflow {cnt.max()}"
        if esum.max() > cap_edges:
            blk = blk.copy()
            for b in np.where(esum > cap_edges)[0]:
                members = np.where(blk == b)[0]
                members = members[np.argsort(deg_in[nodes[members]])]
                k = 0
                while esum[b] > cap_edges and k < len(members):
                    mv = members[k]
                    d = deg_in[nodes[mv]]
                    cands = np.where((esum + d <= cap_edges) & (cnt < RB))[0]
                    if len(cands) == 0:
                        raise RuntimeError("bin packing failed; raise cpb")
                    tgt = cands[np.argmin(esum[cands])]
                    blk[mv] = tgt
                    esum[b] -= d
                    esum[tgt] += d
                    cnt[b] -= 1
                    cnt[tgt] += 1
                    k += 1
            assert esum.max() <= cap_edges
        slot = np.zeros(m, dtype=np.int64)
        so = np.argsort(blk, kind="stable")
        sb = blk[so]
        start = np.r_[0, np.flatnonzero(np.diff(sb)) + 1]
        sizes = np.diff(np.r_[start, m])
        ranks = np.arange(m) - np.repeat(start, sizes)
        slot[so] = ranks
        node_block[nodes] = blk
        node_slot[nodes] = slot
    return node_core, node_block, node_slot


def preprocess(cfg, feat, row, col, edge_weight, W1, b1, W2, b2):
    from concourse import mybir
    bf16 = mybir.dt.np(mybir.dt.bfloat16)
    n = cfg.n
    ncores, nb, cpb = cfg.ncores, cfg.blocks, cfg.cpb
    deg_in = np.bincount(row, minlength=n)
    node_core, node_block, node_slot = _assign_nodes(cfg, deg_in)
    newid = node_core * cfg.bp + node_block * RB + node_slot

    # ---- per (core, block): edges sorted by col ----
    ec_new = newid[col]
    e_core = node_core[row]
    e_blk = node_block[row]
    e_slot = node_slot[row]
    gblk = e_core * nb + e_blk
    so = np.lexsort((ec_new, gblk))
    gblk_s = gblk[so]
    ec_s = ec_new[so]
    slot_s = e_slot[so]
    w_s = edge_weight[so]

    nblk_g = ncores * nb
    blk_cnt = np.bincount(gblk_s, minlength=nblk_g)
    cap_edges = cpb * P
    assert blk_cnt.max() <= cap_edges, f"{blk_cnt.max()} > {cap_edges}"
    blk_start = np.r_[0, np.cumsum(blk_cnt)[:-1]]
    rank_in_blk = np.arange(len(so)) - np.repeat(blk_start, blk_cnt)

    # chunk-level min/max col per (core, block, k) -> cross-core union span
    big = np.int64(1 << 60)
    e_chunk = rank_in_blk // P                      # k within block
    key = gblk_s * cpb + e_chunk                    # (core, block, k)
    cmin = np.full(nblk_g * cpb, big, dtype=np.int64)
    cmax = np.full(nblk_g * cpb, -1, dtype=np.int64)
    np.minimum.at(cmin, key, ec_s)
    np.maximum.at(cmax, key, ec_s)
    u_min = cmin.reshape(ncores, nb, cpb).min(axis=0)   # [nb, cpb]
    u_max = cmax.reshape(ncores, nb, cpb).max(axis=0)

    # ---- shared schedule: per super-block, order chunks by col band and
    # pack into calls with cross-core span <= WIN and <= maxg chunks ----
    chunk_order = []                                # (block, k) stream order
    call_start, call_len, call_base = [], [], []
    t = 0
    for s in range(cfg.nsb):
        items = []
        for b in range(s * cfg.sbb, (s + 1) * cfg.sbb):
            for k in range(cpb):
                mn = u_min[b, k]
                items.append((int(mn) if mn != big else 0,
                              int(u_max[b, k]), b, k))
        items.sort()
        cur0, curn, lo, hi = t, 0, big, -1
        for mn, mx, b, k in items:
            nlo = min(lo, mn) if mx >= 0 else lo
            nhi = max(hi, mx)
            span_ok = (nhi < 0) or (nlo == big) or (nhi - nlo <= WIN - 1)
            if curn > 0 and (curn >= cfg.maxg or not span_ok):
                call_start.append(cur0)
                call_len.append(curn)
                call_base.append(0 if lo == big else
                                 min(int(lo), max(cfg.ntot - WIN, 0)))
                cur0, curn, lo, hi = t, 0, big, -1
                nlo = mn if mx >= 0 else big
                nhi = mx
            lo, hi = nlo, nhi
            chunk_order.append((b, k))
            curn += 1
            t += 1
        call_start.append(cur0)
        call_len.append(curn)
        call_base.append(0 if lo == big else
                         min(int(lo), max(cfg.ntot - WIN, 0)))
    assert t == cfg.totch
    call_start = np.asarray(call_start)
    call_len = np.asarray(call_len)
    call_base = np.asarray(call_base, dtype=np.int64)
    call_width = np.minimum(WIN, cfg.ntot - call_base)

    # chunk meta in stream order
    chunk_block = np.asarray([b for b, _ in chunk_order])
    ck = np.asarray([k for _, k in chunk_order])
    stream_of = np.empty((nb, cpb), dtype=np.int64)
    stream_of[chunk_block, ck] = np.arange(cfg.totch)
    first = np.full(nb, cfg.totch, dtype=np.int64)
    last = np.full(nb, -1, dtype=np.int64)
    np.minimum.at(first, chunk_block, np.arange(cfg.totch))
    np.maximum.at(last, chunk_block, np.arange(cfg.totch))
    chunk_start = np.zeros(cfg.totch, dtype=bool)
    chunk_stop = np.zeros(cfg.totch, dtype=bool)
    chunk_start[first] = True
    chunk_stop[last] = True
    chunk_base = np.repeat(call_base, call_len)

    # ---- per-core padded planes in stream order ----
    tot = cfg.totch * P
    stream_pos = stream_of[gblk_s % nb, e_chunk] * P + rank_in_blk % P
    core_of_edge = gblk_s // nb
    colpad = np.zeros((ncores, tot), dtype=np.int64)
    slotpad = np.zeros((ncores, tot), dtype=np.int64)
    wpad = np.zeros((ncores, tot), dtype=np.float32)
    validpad = np.zeros((ncores, tot), dtype=bool)
    colpad[core_of_edge, stream_pos] = ec_s
    slotpad[core_of_edge, stream_pos] = slot_s
    wpad[core_of_edge, stream_pos] = w_s
    validpad[core_of_edge, stream_pos] = True

    cb_e = np.broadcast_to(np.repeat(chunk_base, P), (ncores, tot))
    colpad = np.where(validpad, colpad, cb_e)
    wpad = np.where(validpad, wpad, 0.0)
    wmax = int(np.where(validpad, colpad - cb_e, 0).max())
    assert wmax < WIN, f"window overflow {wmax}"
    assert (colpad >= cb_e).all()

    idx16 = (colpad - cb_e).astype(np.int16).reshape(ncores, cfg.totch, P)
    assert (idx16 >= 0).all()
    idxp = idx16.reshape(ncores, cfg.totch, 8, 16)
    idxp = idxp.transpose(0, 3, 1, 2).reshape(ncores, 16, cfg.totch * 8)
    idx_plane = np.tile(idxp, (1, 8, 1))

    seg_plane = slotpad.reshape(ncores, cfg.totch, P).transpose(0, 2, 1)
    seg_plane = np.ascontiguousarray(seg_plane).astype(bf16)
    w_plane = wpad.reshape(ncores, cfg.totch, P).transpose(0, 2, 1)
    w_plane = np.ascontiguousarray(w_plane).astype(bf16)

    # iota plane [128, RB*sg] bf16: [p, s*sg + j] = s  (sg = S-gen group)
    iota_plane = np.repeat(np.arange(RB), SG).astype(bf16)
    iota_plane = np.tile(iota_plane, (P, 1))

    # full featT [2, 128, ntot] (replicated to every core)
    fdt = mybir.dt.np(mybir.dt.float8e4) if FP8_FEAT else bf16
    feat_pad = np.zeros((cfg.ntot, F_IN), dtype=np.float32)
    feat_pad[newid] = feat
    featT = np.ascontiguousarray(
        feat_pad.reshape(cfg.ntot, 2, P).transpose(1, 2, 0)).astype(fdt)

    w1p = np.ascontiguousarray(W1.reshape(2, P, F_HID)).astype(bf16)
    w2p = np.ascontiguousarray(W2.reshape(2, P, F_OUT)).astype(bf16)
    b1p = np.ascontiguousarray(b1.reshape(2, P, 1)).astype(np.float32)
    b2p = np.ascontiguousarray(b2.reshape(P, 1)).astype(np.float32)

    in_maps = []
    for c in range(ncores):
        in_maps.append({
            "featT": featT,
            "w1": w1p, "w2": w2p, "b1": b1p, "b2": b2p,
            "iota": iota_plane,
            "idxs": np.ascontiguousarray(idx_plane[c]),
            "segid": seg_plane[c],
            "wgt": w_plane[c],
        })
    meta = {
        "call_start": call_start, "call_len": call_len,
        "base": call_base, "width": call_width,
        "chunk_block": chunk_block, "chunk_start": chunk_start,
        "chunk_stop": chunk_stop,
        "newid": newid, "node_core": node_core,
        "node_block": node_block, "node_slot": node_slot,
    }
    return in_maps, meta


def assemble(cfg, meta, outs):
    """outs: per core {'outT': [nsb, F_OUT, sbb*RB]} -> [n, F_OUT] f32."""
    res = np.empty((cfg.n, F_OUT), dtype=np.float32)
    nc_, nb_, ns_ = meta["node_core"], meta["node_block"], meta["node_slot"]
    for c in range(cfg.ncores):
        o = outs[c]["outT"]
        sel = np.where(nc_ == c)[0]
        b = nb_[sel]
        res[sel] = o[b // cfg.sbb, :, (b % cfg.sbb) * RB + ns_[sel]]
    return res


# --------------------------------------------------------------------------
# Device program
# --------------------------------------------------------------------------

def outer_bcast(ap, k):
    """Insert a step-0 dim of size k before the last dim."""
    from concourse import bass
    a = list(ap.ap)
    return bass.AP(ap.tensor, ap.offset, a[:-1] + [[0, k]] + a[-1:])


def strided_cols(ap, start, step, count):
    """Column view [p, start + step*j] of a contiguous [128, N] AP."""
    from concourse import bass
    a = list(ap.ap)
    return bass.AP(ap.tensor, ap.offset + start, [a[0], [step, count]])


def sgrp_view(tile_ap, maxg, nch, rb):
    """AP [p, s*maxg + j], s in [0, rb), j in [0, nch) of a [128, rb*maxg]
    contiguous tile (last dim packed -> DVE 2x mode)."""
    from concourse import bass
    a = list(tile_ap.ap)
    return bass.AP(tile_ap.tensor, tile_ap.offset, [a[0], [maxg, rb], [1, nch]])


def rows_view(dram_ap, r0, nrows, fel):
    """DRAM view of dram[r0:r0+nrows, :] ordered [p, j, f] with
    row = r0 + j*128 + p, matching an SBUF tile [128, nrows//128, fel]."""
    from concourse import bass
    a = [list(d) for d in dram_ap.ap]
    rs = a[0][0]                       # row stride (elements)
    assert a[1] == [1, fel], a
    return bass.AP(dram_ap.tensor, dram_ap.offset + r0 * rs,
                   [[rs, P], [rs * P, nrows // P], [1, fel]])


def build_program(tc, cfg, meta, outs, ins):
    import os as _os
    from concourse import mybir
    nc = tc.nc
    dt = mybir.dt
    base, width = meta["base"], meta["width"]
    call_start, call_len = meta["call_start"], meta["call_len"]
    chunk_block = meta["chunk_block"]
    chunk_start, chunk_stop = meta["chunk_start"], meta["chunk_stop"]
    featT, w1, w2 = ins["featT"], ins["w1"], ins["w2"]
    b1, b2 = ins["b1"], ins["b2"]
    iota, idxs, segid, wgt = ins["iota"], ins["idxs"], ins["segid"], ins["wgt"]
    outT = outs["outT"]
    SBB, MAXG = cfg.sbb, cfg.maxg
    SBW = SBB * RB                          # psum cols per super-block (512)
    SBCH = SBB * cfg.cpb                    # chunks per super-block
    rg = [list(range(cfg.ncores))]
    calls = len(call_start)

    with tc.tile_pool(name="const", bufs=1) as const, \
         tc.tile_pool(name="dram", bufs=1, space="DRAM") as dram, \
         tc.tile_pool(name="gpool", bufs=3) as gpool, \
         tc.tile_pool(name="spool", bufs=3) as spool, \
         tc.tile_pool(name="hpool", bufs=2) as hpool, \
         tc.tile_pool(name="ypool", bufs=2) as ypool:
        # ------- constants -------
        idx_sb = const.tile([P, cfg.totch * 8], dt.int16)
        nc.sync.dma_start(idx_sb[:], idxs[:])
        seg_sb = const.tile([P, cfg.totch], dt.bfloat16)
        nc.sync.dma_start(seg_sb[:], segid[:])
        w_sb = const.tile([P, cfg.totch], dt.bfloat16)
        nc.sync.dma_start(w_sb[:], wgt[:])
        iota_sb = const.tile([P, RB * SG], dt.bfloat16)
        nc.sync.dma_start(iota_sb[:], iota[:])
        w1_sb = const.tile([P, 2 * F_HID], dt.bfloat16)
        nc.sync.dma_start(w1_sb[:, 0:F_HID], w1[0])
        nc.sync.dma_start(w1_sb[:, F_HID:2 * F_HID], w1[1])
        w2_sb = const.tile([P, 2 * F_OUT], dt.bfloat16)
        nc.sync.dma_start(w2_sb[:, 0:F_OUT], w2[0])
        nc.sync.dma_start(w2_sb[:, F_OUT:2 * F_OUT], w2[1])
        b1_sb = const.tile([P, 2], dt.float32)
        nc.sync.dma_start(b1_sb[:, 0:1], b1[0])
        nc.sync.dma_start(b1_sb[:, 1:2], b1[1])
        b2_sb = const.tile([P, 1], dt.float32)
        nc.sync.dma_start(b2_sb[:], b2[:])

        zdt = dt.float8e4 if FP8_Z else dt.bfloat16
        fdt = dt.float8e4 if FP8_FEAT else dt.bfloat16
        zall = dram.tile([cfg.ntot, F_HID], zdt)
        yin = dram.tile([cfg.bp, F_OUT], dt.bfloat16)
        yall = dram.tile([cfg.ntot, F_OUT], dt.bfloat16, addr_space="Shared")

        # ------- phase Z: full Z = feat @ W1 on every core -------
        ZST = 2048                          # rows per feat supertile
        assert cfg.ntot % ZST == 0
        SKIP_Z = bool(_os.environ.get("SKIP_Z"))
        with tc.tile_pool(name="ftpool", bufs=2) as ftpool, \
             tc.tile_pool(name="zpool", bufs=3) as zpool, \
             tc.tile_pool(name="psum_z", bufs=2, space="PSUM") as psum_z:
            NM = ZST // P                   # matmuls per supertile (16)
            for g in range(cfg.ntot // ZST if not SKIP_Z else 0):
                ft = ftpool.tile([P, 2, ZST], fdt, tag="ft")
                nc.sync.dma_start(ft[:, 0, :], featT[0, :, g * ZST:(g + 1) * ZST])
                nc.sync.dma_start(ft[:, 1, :], featT[1, :, g * ZST:(g + 1) * ZST])
                zb = zpool.tile([P, NM, F_HID], zdt, tag="zb")
                for j in range(NM // 4):         # 4 node-interleaved matmuls
                    pz = psum_z.tile([P, 4, F_HID], dt.float32,
                                     space="PSUM", tag="pz")
                    for jj in range(4):
                        m = j * 4 + jj
                        # out partition p holds node g*ZST + NM*p + m so a
                        # partition's NM rows are consecutive in zall
                        for h in range(2):
                            nc.tensor.matmul(
                                out=pz[:, jj, :],
                                lhsT=strided_cols(ft[:, h, :], m, NM, P),
                                rhs=w1_sb[:, h * F_HID:(h + 1) * F_HID],
                                start=(jj % 2 == 0 and h == 0),
                                stop=(jj % 2 == 1 and h == 1))
                    zs = zb[:, j * 4:(j + 1) * 4, :]
                    if j % 2 == 0:
                        nc.vector.tensor_copy(zs, pz[:, :, :])
                    else:
                        nc.scalar.activation(
                            zs, pz[:, :, :],
                            mybir.ActivationFunctionType.Copy)
                # contiguous multi-row descriptors per partition; 8KB
                # descriptors hang the DGE, so cap at 4KB per descriptor
                from concourse import bass
                default_split = "1" if FP8_Z else "2"
                nsplit = int(_os.environ.get("GCN_ZSPLIT", default_split))
                tot = F_HID * NM
                zv = zall[g * ZST:(g + 1) * ZST, :]
                dims = [[tot, P]]
                if nsplit > 1:
                    dims.append([tot // nsplit, nsplit])
                dims.append([1, tot // nsplit])
                dst = bass.AP(zv.tensor, zv.offset, dims)
                nc.sync.dma_start(dst, zb[:])

        # ------- shared gather/aggregate layer -------
        psum_ctx = tc.tile_pool(name="psum_h", bufs=2, space="PSUM")
        psum_h = psum_ctx.__enter__()
        psum_y_ctx = tc.tile_pool(name="psum_y", bufs=2, space="PSUM")
        psum_y = psum_y_ctx.__enter__()

        def layer(src, felem, gdt, nhalves, out_cb):
            ph = {}                         # super-block -> psum tiles
            sgen = {}                       # S-gen group -> S tile

            def get_S(g):
                if g in sgen:
                    return sgen[g]
                c0g = g * SG
                eq = spool.tile([P, RB * SG], dt.bfloat16, tag="eq",
                                name="eq")
                S = spool.tile([P, RB * SG], dt.bfloat16, tag="S", name="S")
                nc.vector.tensor_tensor(
                    out=eq[:], in0=iota_sb[:],
                    in1=outer_bcast(seg_sb[:, c0g:c0g + SG], RB),
                    op=mybir.AluOpType.is_equal)
                nc.vector.tensor_tensor(
                    out=S[:], in0=eq[:],
                    in1=outer_bcast(w_sb[:, c0g:c0g + SG], RB),
                    op=mybir.AluOpType.mult)
                sgen[g] = S
                return S

            for tcall in range(calls):
                c0, nch = int(call_start[tcall]), int(call_len[tcall])
                gt = gpool.tile([P, MAXG, felem], gdt, tag="gt")
                win = src[int(base[tcall]):int(base[tcall] + width[tcall]), :]
                if not _os.environ.get("SKIP_GATHER"):
                    nc.gpsimd.dma_gather(
                        gt[:, :nch, :], win, idx_sb[:, c0 * 8:(c0 + nch) * 8],
                        nch * P, nch * P, felem, elem_step=felem,
                        queue_num=(tcall % NQ))
                else:
                    nc.vector.tensor_copy(gt[:, 0, 0:P], seg_sb[:, 0:P])
                for j in range(nch):
                    c = c0 + j
                    b = int(chunk_block[c])
                    sb = b // SBB
                    S = get_S(c // SG)
                    if sb not in ph:
                        ph[sb] = [psum_h.tile([P, SBW], dt.float32,
                                              space="PSUM", tag=f"ph{h}",
                                              name=f"ph{h}")
                                  for h in range(nhalves)]
                    col0 = (b % SBB) * RB
                    # one psum group per bank per super-block (the start
                    # marks the whole 2KB zero region; slices lazily zero
                    # on first write)
                    for h in range(nhalves):
                        nc.tensor.matmul(
                            out=ph[sb][h][:, col0:col0 + RB],
                            lhsT=gt[:, j, h * P:(h + 1) * P],
                            rhs=strided_cols(S[:], c % SG, SG, RB),
                            start=(c % SBCH == 0),
                            stop=(c % SBCH == SBCH - 1))
                    if c == (sb + 1) * SBCH - 1:
                        out_cb(sb, ph.pop(sb))

        # ------- layer 1 output: relu + bias, Y = H @ W2 -------
        def l1_out(sb, phs):
            from concourse import bass
            ht = hpool.tile([P, 2, SBW], dt.bfloat16, tag="ht")
            for h in range(2):
                nc.scalar.activation(
                    ht[:, h, :], phs[h][:],
                    mybir.ActivationFunctionType.Relu,
                    bias=b1_sb[:, h:h + 1])
            # 4 node-interleaved matmuls: py partition p = SB row 4p + m,
            # so each partition's 4 Y rows are consecutive in yin
            py = psum_y.tile([P, 4, F_OUT], dt.float32, space="PSUM",
                             tag="py")
            for m in range(4):
                for h in range(2):
                    nc.tensor.matmul(
                        out=py[:, m, :],
                        lhsT=strided_cols(ht[:, h, :], m, 4, P),
                        rhs=w2_sb[:, h * F_OUT:(h + 1) * F_OUT],
                        start=(m == 0 and h == 0),
                        stop=(m == 3 and h == 1))
            yb = ypool.tile([P, 4 * F_OUT], dt.bfloat16, tag="yb")
            nc.vector.tensor_copy(yb[:], py[:, :, :])
            yv = yin[sb * SBW:(sb + 1) * SBW, :]
            dst = bass.AP(yv.tensor, yv.offset,
                          [[4 * F_OUT, P], [1, 4 * F_OUT]])
            nc.sync.dma_start(dst, yb[:])

        layer(zall[:], F_IN, zdt, 2, l1_out)

        if cfg.ncores > 1 and not _os.environ.get("TINY_AG"):
            nc.gpsimd.collective_compute(
                "AllGather", mybir.AluOpType.bypass, replica_groups=rg,
                ins=[yin.opt()], outs=[yall.opt()])
        else:
            nc.sync.dma_start(yall[0:cfg.bp, :], yin[:])

        # ------- layer 2 output -------
        def l2_out(sb, phs):
            ob = hpool.tile([P, SBW], dt.float32, tag="ob")
            nc.scalar.activation(
                ob[:], phs[0][:], mybir.ActivationFunctionType.Identity,
                bias=b2_sb[:, 0:1])
            # outT is [nsb, F_OUT, SBW]: one 2KB-per-partition write per SB
            nc.sync.dma_start(outT[sb], ob[:])

        layer(yall[:], F_OUT, dt.bfloat16, 1, l2_out)
        psum_y_ctx.__exit__(None, None, None)
        psum_ctx.__exit__(None, None, None)


# --------------------------------------------------------------------------
# Top level
# --------------------------------------------------------------------------

def declare_io(nc, cfg):
    from concourse import mybir
    dt = mybir.dt

    def di(name, shape, d):
        return nc.dram_tensor(name, shape, d, kind="ExternalInput").ap()
    ins = {
        "featT": di("featT", [2, P, cfg.ntot],
                    dt.float8e4 if FP8_FEAT else dt.bfloat16),
        "w1": di("w1", [2, P, F_HID], dt.bfloat16),
        "w2": di("w2", [2, P, F_OUT], dt.bfloat16),
        "b1": di("b1", [2, P, 1], dt.float32),
        "b2": di("b2", [P, 1], dt.float32),
        "iota": di("iota", [P, RB * SG], dt.bfloat16),
        "idxs": di("idxs", [P, cfg.totch * 8], dt.int16),
        "segid": di("segid", [P, cfg.totch], dt.bfloat16),
        "wgt": di("wgt", [P, cfg.totch], dt.bfloat16),
    }
    outs = {
        "outT": nc.dram_tensor("outT", [cfg.nsb, F_OUT, cfg.sbb * RB],
                               dt.float32, kind="ExternalOutput").ap(),
    }
    return ins, outs


def build_nc(cfg, meta, repeat=1):
    import sys
    sys.path.insert(0, "/opt/trn_rl_repo")
    from concourse import bacc, tile
    scratch = int(_os_mod.environ.get("GCN_DMA_SCRATCH", "16384"))
    nc = bacc.Bacc("TRN2", target_bir_lowering=False, debug=False,
                   num_devices=cfg.ncores, num_swdge_queues=NQ,
                   dynamic_dma_scratch_size=scratch)
    ins, outs = declare_io(nc, cfg)
    with tile.TileContext(nc) as tc:
        for _ in range(repeat):
            build_program(tc, cfg, meta, outs, ins)
    nc.compile()
    return nc


def kernel(feat, row, col, edge_weight, W1, b1, W2, b2):
    import sys
    sys.path.insert(0, "/opt/trn_rl_repo")
    feat = np.asarray(feat, dtype=np.float32)
    row = np.asarray(row, dtype=np.int32)
    col = np.asarray(col, dtype=np.int32)
    edge_weight = np.asarray(edge_weight, dtype=np.float32)
    W1 = np.asarray(W1, dtype=np.float32)
    b1 = np.asarray(b1, dtype=np.float32)
    W2 = np.asarray(W2, dtype=np.float32)
    b2 = np.asarray(b2, dtype=np.float32)

    cfg = full_cfg()
    assert feat.shape == (cfg.n, F_IN) and row.shape == (cfg.e,)

    in_maps, meta = preprocess(cfg, feat, row, col, edge_weight, W1, b1, W2, b2)
    nc = build_nc(cfg, meta)

    from concourse.bass_utils import run_bass_kernel_spmd
    res = run_bass_kernel_spmd(nc, in_maps, core_ids=list(range(cfg.ncores)))
    outs = [{"outT": r["outT"]} for r in res.results]
    return assemble(cfg, meta, outs)



# revision 15
# speedup vs baseline: 2.0139x; 2.0139x over previous
"""2-layer GCN (SpMM message passing) on 8 Trainium2 NeuronCores — v3.

Strategy (dest-row-sharded SpMM, fused layer-1):
  - Dest nodes are relabeled to (core, block, slot): 8 cores x BLOCKS blocks
    x 64 slots.  Per-core bin packing (snake by degree + repair) bounds each
    block's in-edge count by CPB*128.
  - Layer 1 is fused: since A@(X@W1) == (A@X)@W1, each core gathers raw
    fp8 feat rows per edge, aggregates A@X per super-block in PSUM, then
    applies W1 to the tiny [256, 512] aggregate (no replicated full-Z
    matmul phase, no Z DRAM round-trip).
  - Per core, edges are grouped by dest block and sorted by source col;
    the 128-edge chunks of a super-block (8 blocks) are interleaved by col
    band and packed into dma_gather calls of up to MAXG chunks whose
    cross-core col span fits the int16 window (32767 rows).
  - Aggregation: per chunk j, matmul lhsT=gathered[128 edges, feat-half],
    rhs=S cols (one-hot(slot)*w built on DVE per call) accumulating into
    the chunk's block slice of a [128, 512] PSUM bank per feat half.
  - Per super-block: agg -> bf16, H^T = relu(W1^T@agg + b1), Y = H @ W2
    per 128-row pair, then AllGather Y and layer 2 runs the same schedule
    on yall.
"""

import numpy as np

P = 128
RB = 64            # rows (slots) per block
F_IN = 256
F_HID = 256
F_OUT = 128
WIN = 32767        # dma_gather int16 index window
NQ = 4             # SWDGE queues for gathers (ucode max)
import os as _os_mod
FP8_FEAT = _os_mod.environ.get("GCN_FP8_FEAT", "1") == "1"
SG = 32            # chunks per S-matrix generation group (amortizes DVE ops)


class Cfg:
    def __init__(self, n_nodes, n_edges, ncores, cpb, maxg=48, sbb=8):
        assert n_nodes % ncores == 0
        self.n = n_nodes
        self.e = n_edges
        self.ncores = ncores
        self.npc = n_nodes // ncores
        nb = (self.npc + RB - 1) // RB
        if nb * RB - self.npc < 32:      # slack rows for bin packing
            nb += 1
        if nb % sbb:
            nb += sbb - nb % sbb         # whole super-blocks
        self.blocks = nb
        self.sbb = sbb                   # blocks per super-block
        self.nsb = nb // sbb
        self.cpb = cpb                   # chunks (of 128 edges) per block
        self.maxg = maxg                 # max chunks per dma_gather call
        self.totch = nb * cpb
        self.bp = nb * RB                # padded nodes per core
        self.ntot = ncores * self.bp


def full_cfg():
    # per-block capacity cpb*128=2048 vs ~2000 mean edges; bin packing keeps
    # every block under cap (assert-checked).
    import os
    maxg = int(os.environ.get("GCN_MAXG", "8"))  # >8 overflows the Q7 idx scratch and hangs
    return Cfg(100000, 3200000, 8, cpb=16, maxg=maxg, sbb=8)


# --------------------------------------------------------------------------
# Host-side preprocessing
# --------------------------------------------------------------------------

def _assign_nodes(cfg, deg_in):
    """node -> (core, block, slot), balancing per-core and per-block edges."""
    n, ncores, nb = cfg.n, cfg.ncores, cfg.blocks
    order = np.argsort(-deg_in, kind="stable")
    pos = np.arange(n)
    phase = pos % (2 * ncores)
    core_of_pos = np.where(phase < ncores, phase, 2 * ncores - 1 - phase)
    node_core = np.empty(n, dtype=np.int64)
    node_core[order] = core_of_pos

    node_block = np.empty(n, dtype=np.int64)
    node_slot = np.empty(n, dtype=np.int64)
    cap_edges = cfg.cpb * P
    for c in range(ncores):
        nodes = order[core_of_pos == c]          # degree-desc within core
        m = len(nodes)
        assert m == cfg.npc
        bpos = np.arange(m)
        ph = bpos % (2 * nb)
        blk = np.where(ph < nb, ph, 2 * nb - 1 - ph)
        cnt = np.bincount(blk, minlength=nb)
        esum = np.bincount(blk, weights=deg_in[nodes], minlength=nb)
        assert cnt.max() <= RB, f"block row overflow {cnt.max()}"
        if esum.max() > cap_edges:
            blk = blk.copy()
            for b in np.where(esum > cap_edges)[0]:
                members = np.where(blk == b)[0]
                members = members[np.argsort(deg_in[nodes[members]])]
                k = 0
                while esum[b] > cap_edges and k < len(members):
                    mv = members[k]
                    d = deg_in[nodes[mv]]
                    cands = np.where((esum + d <= cap_edges) & (cnt < RB))[0]
                    if len(cands) == 0:
                        raise RuntimeError("bin packing failed; raise cpb")
                    tgt = cands[np.argmin(esum[cands])]
                    blk[mv] = tgt
                    esum[b] -= d
                    esum[tgt] += d
                    cnt[b] -= 1
                    cnt[tgt] += 1
                    k += 1
            assert esum.max() <= cap_edges
        slot = np.zeros(m, dtype=np.int64)
        so = np.argsort(blk, kind="stable")
        sb = blk[so]
        start = np.r_[0, np.flatnonzero(np.diff(sb)) + 1]
        sizes = np.diff(np.r_[start, m])
        ranks = np.arange(m) - np.repeat(start, sizes)
        slot[so] = ranks
        node_block[nodes] = blk
        node_slot[nodes] = slot
    return node_core, node_block, node_slot


def preprocess(cfg, feat, row, col, edge_weight, W1, b1, W2, b2):
    from concourse import mybir
    bf16 = mybir.dt.np(mybir.dt.bfloat16)
    n = cfg.n
    ncores, nb, cpb = cfg.ncores, cfg.blocks, cfg.cpb
    deg_in = np.bincount(row, minlength=n)
    node_core, node_block, node_slot = _assign_nodes(cfg, deg_in)
    newid = node_core * cfg.bp + node_block * RB + node_slot

    # ---- per (core, block): edges sorted by col ----
    ec_new = newid[col]
    e_core = node_core[row]
    e_blk = node_block[row]
    e_slot = node_slot[row]
    gblk = e_core * nb + e_blk
    so = np.lexsort((ec_new, gblk))
    gblk_s = gblk[so]
    ec_s = ec_new[so]
    slot_s = e_slot[so]
    w_s = edge_weight[so]

    nblk_g = ncores * nb
    blk_cnt = np.bincount(gblk_s, minlength=nblk_g)
    cap_edges = cpb * P
    assert blk_cnt.max() <= cap_edges, f"{blk_cnt.max()} > {cap_edges}"
    blk_start = np.r_[0, np.cumsum(blk_cnt)[:-1]]
    rank_in_blk = np.arange(len(so)) - np.repeat(blk_start, blk_cnt)

    # chunk-level min/max col per (core, block, k) -> cross-core union span
    big = np.int64(1 << 60)
    e_chunk = rank_in_blk // P                      # k within block
    key = gblk_s * cpb + e_chunk                    # (core, block, k)
    cmin = np.full(nblk_g * cpb, big, dtype=np.int64)
    cmax = np.full(nblk_g * cpb, -1, dtype=np.int64)
    np.minimum.at(cmin, key, ec_s)
    np.maximum.at(cmax, key, ec_s)
    u_min = cmin.reshape(ncores, nb, cpb).min(axis=0)   # [nb, cpb]
    u_max = cmax.reshape(ncores, nb, cpb).max(axis=0)

    # ---- shared schedule: per super-block, order chunks by col band and
    # pack into calls with cross-core span <= WIN and <= maxg chunks ----
    chunk_order = []                                # (block, k) stream order
    call_start, call_len, call_base = [], [], []
    t = 0
    for s in range(cfg.nsb):
        items = []
        for b in range(s * cfg.sbb, (s + 1) * cfg.sbb):
            for k in range(cpb):
                mn = u_min[b, k]
                items.append((int(mn) if mn != big else 0,
                              int(u_max[b, k]), b, k))
        items.sort()
        cur0, curn, lo, hi = t, 0, big, -1
        for mn, mx, b, k in items:
            nlo = min(lo, mn) if mx >= 0 else lo
            nhi = max(hi, mx)
            span_ok = (nhi < 0) or (nlo == big) or (nhi - nlo <= WIN - 1)
            if curn > 0 and (curn >= cfg.maxg or not span_ok):
                call_start.append(cur0)
                call_len.append(curn)
                call_base.append(0 if lo == big else
                                 min(int(lo), max(cfg.ntot - WIN, 0)))
                cur0, curn, lo, hi = t, 0, big, -1
                nlo = mn if mx >= 0 else big
                nhi = mx
            lo, hi = nlo, nhi
            chunk_order.append((b, k))
            curn += 1
            t += 1
        call_start.append(cur0)
        call_len.append(curn)
        call_base.append(0 if lo == big else
                         min(int(lo), max(cfg.ntot - WIN, 0)))
    assert t == cfg.totch
    call_start = np.asarray(call_start)
    call_len = np.asarray(call_len)
    call_base = np.asarray(call_base, dtype=np.int64)
    call_width = np.minimum(WIN, cfg.ntot - call_base)

    # chunk meta in stream order
    chunk_block = np.asarray([b for b, _ in chunk_order])
    ck = np.asarray([k for _, k in chunk_order])
    stream_of = np.empty((nb, cpb), dtype=np.int64)
    stream_of[chunk_block, ck] = np.arange(cfg.totch)
    first = np.full(nb, cfg.totch, dtype=np.int64)
    last = np.full(nb, -1, dtype=np.int64)
    np.minimum.at(first, chunk_block, np.arange(cfg.totch))
    np.maximum.at(last, chunk_block, np.arange(cfg.totch))
    chunk_start = np.zeros(cfg.totch, dtype=bool)
    chunk_stop = np.zeros(cfg.totch, dtype=bool)
    chunk_start[first] = True
    chunk_stop[last] = True
    chunk_base = np.repeat(call_base, call_len)

    # ---- per-core padded planes in stream order ----
    tot = cfg.totch * P
    stream_pos = stream_of[gblk_s % nb, e_chunk] * P + rank_in_blk % P
    core_of_edge = gblk_s // nb
    colpad = np.zeros((ncores, tot), dtype=np.int64)
    slotpad = np.zeros((ncores, tot), dtype=np.int64)
    wpad = np.zeros((ncores, tot), dtype=np.float32)
    validpad = np.zeros((ncores, tot), dtype=bool)
    colpad[core_of_edge, stream_pos] = ec_s
    slotpad[core_of_edge, stream_pos] = slot_s
    wpad[core_of_edge, stream_pos] = w_s
    validpad[core_of_edge, stream_pos] = True

    cb_e = np.broadcast_to(np.repeat(chunk_base, P), (ncores, tot))
    colpad = np.where(validpad, colpad, cb_e)
    wpad = np.where(validpad, wpad, 0.0)
    wmax = int(np.where(validpad, colpad - cb_e, 0).max())
    assert wmax < WIN, f"window overflow {wmax}"
    assert (colpad >= cb_e).all()

    idx16 = (colpad - cb_e).astype(np.int16).reshape(ncores, cfg.totch, P)
    assert (idx16 >= 0).all()
    if _os_mod.environ.get("GCN_SEQ_IDX"):
        # diagnostic: sequential in-window indices (perfect HBM locality,
        # same descriptor count). Results are garbage; timing-only.
        cs = np.repeat(call_start, call_len)
        seq = ((np.arange(cfg.totch) - cs)[:, None] * P +
               np.arange(P)[None, :]).astype(np.int16)
        idx16 = np.broadcast_to(seq[None], (ncores, cfg.totch, P)).copy()
    idxp = idx16.reshape(ncores, cfg.totch, 8, 16)
    idxp = idxp.transpose(0, 3, 1, 2).reshape(ncores, 16, cfg.totch * 8)
    idx_plane = np.tile(idxp, (1, 8, 1))

    seg_plane = slotpad.reshape(ncores, cfg.totch, P).transpose(0, 2, 1)
    seg_plane = np.ascontiguousarray(seg_plane).astype(bf16)
    w_plane = wpad.reshape(ncores, cfg.totch, P).transpose(0, 2, 1)
    w_plane = np.ascontiguousarray(w_plane).astype(bf16)

    # iota plane [128, RB*sg] bf16: [p, s*sg + j] = s  (sg = S-gen group)
    iota_plane = np.repeat(np.arange(RB), SG).astype(bf16)
    iota_plane = np.tile(iota_plane, (P, 1))

    # full node-major feat [ntot, F_IN] (replicated to every core)
    fdt = mybir.dt.np(mybir.dt.float8e4) if FP8_FEAT else bf16
    feat_pad = np.zeros((cfg.ntot, F_IN), dtype=np.float32)
    feat_pad[newid] = feat
    featN = np.ascontiguousarray(feat_pad).astype(fdt)

    w1p = np.ascontiguousarray(W1.reshape(2, P, F_HID)).astype(bf16)
    w2p = np.ascontiguousarray(W2.reshape(2, P, F_OUT)).astype(bf16)
    b1p = np.ascontiguousarray(b1.reshape(2, P, 1)).astype(np.float32)
    b2p = np.ascontiguousarray(b2.reshape(P, 1)).astype(np.float32)

    in_maps = []
    for c in range(ncores):
        in_maps.append({
            "featN": featN,
            "w1": w1p, "w2": w2p, "b1": b1p, "b2": b2p,
            "iota": iota_plane,
            "idxs": np.ascontiguousarray(idx_plane[c]),
            "segid": seg_plane[c],
            "wgt": w_plane[c],
        })
    meta = {
        "call_start": call_start, "call_len": call_len,
        "base": call_base, "width": call_width,
        "chunk_block": chunk_block, "chunk_start": chunk_start,
        "chunk_stop": chunk_stop,
        "newid": newid, "node_core": node_core,
        "node_block": node_block, "node_slot": node_slot,
    }
    return in_maps, meta


def assemble(cfg, meta, outs):
    """outs: per core {'outT': [nsb, F_OUT, sbb*RB]} -> [n, F_OUT] f32."""
    res = np.empty((cfg.n, F_OUT), dtype=np.float32)
    nc_, nb_, ns_ = meta["node_core"], meta["node_block"], meta["node_slot"]
    for c in range(cfg.ncores):
        o = outs[c]["outT"]
        sel = np.where(nc_ == c)[0]
        b = nb_[sel]
        res[sel] = o[b // cfg.sbb, :, (b % cfg.sbb) * RB + ns_[sel]]
    return res


# --------------------------------------------------------------------------
# Device program
# --------------------------------------------------------------------------

def outer_bcast(ap, k):
    """Insert a step-0 dim of size k before the last dim."""
    from concourse import bass
    a = list(ap.ap)
    return bass.AP(ap.tensor, ap.offset, a[:-1] + [[0, k]] + a[-1:])


def strided_cols(ap, start, step, count):
    """Column view [p, start + step*j] of a contiguous [128, N] AP."""
    from concourse import bass
    a = list(ap.ap)
    return bass.AP(ap.tensor, ap.offset + start, [a[0], [step, count]])


def sgrp_view(tile_ap, maxg, nch, rb):
    """AP [p, s*maxg + j], s in [0, rb), j in [0, nch) of a [128, rb*maxg]
    contiguous tile (last dim packed -> DVE 2x mode)."""
    from concourse import bass
    a = list(tile_ap.ap)
    return bass.AP(tile_ap.tensor, tile_ap.offset, [a[0], [maxg, rb], [1, nch]])


def rows_view(dram_ap, r0, nrows, fel):
    """DRAM view of dram[r0:r0+nrows, :] ordered [p, j, f] with
    row = r0 + j*128 + p, matching an SBUF tile [128, nrows//128, fel]."""
    from concourse import bass
    a = [list(d) for d in dram_ap.ap]
    rs = a[0][0]                       # row stride (elements)
    assert a[1] == [1, fel], a
    return bass.AP(dram_ap.tensor, dram_ap.offset + r0 * rs,
                   [[rs, P], [rs * P, nrows // P], [1, fel]])


def build_program(tc, cfg, meta, outs, ins):
    import os as _os
    from concourse import mybir
    nc = tc.nc
    dt = mybir.dt
    base, width = meta["base"], meta["width"]
    call_start, call_len = meta["call_start"], meta["call_len"]
    chunk_block = meta["chunk_block"]
    chunk_start, chunk_stop = meta["chunk_start"], meta["chunk_stop"]
    featN, w1, w2 = ins["featN"], ins["w1"], ins["w2"]
    b1, b2 = ins["b1"], ins["b2"]
    iota, idxs, segid, wgt = ins["iota"], ins["idxs"], ins["segid"], ins["wgt"]
    outT = outs["outT"]
    SBB, MAXG = cfg.sbb, cfg.maxg
    SBW = SBB * RB                          # psum cols per super-block (512)
    SBCH = SBB * cfg.cpb                    # chunks per super-block
    rg = [list(range(cfg.ncores))]
    calls = len(call_start)

    with tc.tile_pool(name="const", bufs=1) as const, \
         tc.tile_pool(name="dram", bufs=1, space="DRAM") as dram, \
         tc.tile_pool(name="gpool", bufs=int(_os_mod.environ.get(
             "GCN_GBUFS", "6"))) as gpool, \
         tc.tile_pool(name="spool", bufs=4) as spool, \
         tc.tile_pool(name="hpool", bufs=3) as hpool, \
         tc.tile_pool(name="ypool", bufs=3) as ypool:
        # ------- constants -------
        idx_sb = const.tile([P, cfg.totch * 8], dt.int16)
        nc.sync.dma_start(idx_sb[:], idxs[:])
        seg_sb = const.tile([P, cfg.totch], dt.bfloat16)
        nc.sync.dma_start(seg_sb[:], segid[:])
        w_sb = const.tile([P, cfg.totch], dt.bfloat16)
        nc.sync.dma_start(w_sb[:], wgt[:])
        iota_sb = const.tile([P, RB * SG], dt.bfloat16)
        nc.sync.dma_start(iota_sb[:], iota[:])
        w1_sb = const.tile([P, 2 * F_HID], dt.bfloat16)
        nc.sync.dma_start(w1_sb[:, 0:F_HID], w1[0])
        nc.sync.dma_start(w1_sb[:, F_HID:2 * F_HID], w1[1])
        w2_sb = const.tile([P, 2 * F_OUT], dt.bfloat16)
        nc.sync.dma_start(w2_sb[:, 0:F_OUT], w2[0])
        nc.sync.dma_start(w2_sb[:, F_OUT:2 * F_OUT], w2[1])
        b1_sb = const.tile([P, 2], dt.float32)
        nc.sync.dma_start(b1_sb[:, 0:1], b1[0])
        nc.sync.dma_start(b1_sb[:, 1:2], b1[1])
        b2_sb = const.tile([P, 1], dt.float32)
        nc.sync.dma_start(b2_sb[:], b2[:])

        fdt = dt.float8e4 if FP8_FEAT else dt.bfloat16
        yin = dram.tile([cfg.bp, F_OUT], dt.bfloat16)
        yall = dram.tile([cfg.ntot, F_OUT], dt.bfloat16, addr_space="Shared")

        # ------- shared gather/aggregate layer -------
        psum_ctx = tc.tile_pool(name="psum_h", bufs=2, space="PSUM")
        psum_h = psum_ctx.__enter__()
        psum_h2_ctx = tc.tile_pool(name="psum_h2", bufs=1, space="PSUM")
        psum_h2 = psum_h2_ctx.__enter__()
        psum_y_ctx = tc.tile_pool(name="psum_y", bufs=2, space="PSUM")
        psum_y = psum_y_ctx.__enter__()

        def layer(src, felem, gdt, nhalves, out_cb):
            ph = {}                         # super-block -> psum tiles
            sgen = {}                       # S-gen group -> S tile

            def get_S(g):
                if g in sgen:
                    return sgen[g]
                c0g = g * SG
                eq = spool.tile([P, RB * SG], dt.bfloat16, tag="eq",
                                name="eq")
                S = spool.tile([P, RB * SG], dt.bfloat16, tag="S", name="S")
                nc.vector.tensor_tensor(
                    out=eq[:], in0=iota_sb[:],
                    in1=outer_bcast(seg_sb[:, c0g:c0g + SG], RB),
                    op=mybir.AluOpType.is_equal)
                nc.vector.tensor_tensor(
                    out=S[:], in0=eq[:],
                    in1=outer_bcast(w_sb[:, c0g:c0g + SG], RB),
                    op=mybir.AluOpType.mult)
                sgen[g] = S
                return S

            for tcall in range(calls):
                c0, nch = int(call_start[tcall]), int(call_len[tcall])
                gt = gpool.tile([P, MAXG, felem], gdt, tag="gt")
                win = src[int(base[tcall]):int(base[tcall] + width[tcall]), :]
                if not _os.environ.get("SKIP_GATHER"):
                    nc.gpsimd.dma_gather(
                        gt[:, :nch, :], win, idx_sb[:, c0 * 8:(c0 + nch) * 8],
                        nch * P, nch * P, felem, elem_step=felem,
                        queue_num=(tcall % NQ))
                else:
                    nc.vector.tensor_copy(gt[:, 0, 0:P], seg_sb[:, 0:P])
                if _os.environ.get("GCN_SKIP_MM"):
                    continue       # timing ablation: gathers only
                for j in range(nch):
                    c = c0 + j
                    b = int(chunk_block[c])
                    sb = b // SBB
                    S = get_S(c // SG)
                    if sb not in ph:
                        ph[sb] = [psum_h.tile([P, SBW], dt.float32,
                                              space="PSUM", tag=f"ph{h}",
                                              name=f"ph{h}")
                                  for h in range(nhalves)]
                    col0 = (b % SBB) * RB
                    # one psum group per bank per super-block (the start
                    # marks the whole 2KB zero region; slices lazily zero
                    # on first write)
                    for h in range(nhalves):
                        nc.tensor.matmul(
                            out=ph[sb][h][:, col0:col0 + RB],
                            lhsT=gt[:, j, h * P:(h + 1) * P],
                            rhs=strided_cols(S[:], c % SG, SG, RB),
                            start=(c % SBCH == 0),
                            stop=(c % SBCH == SBCH - 1))
                    if c == (sb + 1) * SBCH - 1:
                        out_cb(sb, ph.pop(sb))

        # ------- layer 1 output: agg -> W1 -> relu + bias -> Y = H @ W2 ----
        def l1_out(sb, phs):
            from concourse import bass
            # phs: feat-in-major aggregate [2][128 fin, 512 slots] in PSUM.
            aggT = hpool.tile([P, 2, SBW], dt.bfloat16, tag="aggT")
            for h in range(2):
                nc.scalar.activation(
                    aggT[:, h, :], phs[h][:],
                    mybir.ActivationFunctionType.Copy)
            # H^T[hid, slot] = W1^T @ agg: lhsT = W1[fin, hid] per 128-half
            ph2 = [psum_h2.tile([P, SBW], dt.float32, space="PSUM",
                                tag=f"ph2{h}", name=f"ph2{h}")
                   for h in range(2)]
            for hh in range(2):
                for fh in range(2):
                    nc.tensor.matmul(
                        out=ph2[hh][:],
                        lhsT=w1_sb[:, fh * F_HID + hh * P:
                                   fh * F_HID + hh * P + P],
                        rhs=aggT[:, fh, :],
                        start=(fh == 0), stop=(fh == 1))
            ht = hpool.tile([P, 2, SBW], dt.bfloat16, tag="ht")
            for h in range(2):
                nc.scalar.activation(
                    ht[:, h, :], ph2[h][:],
                    mybir.ActivationFunctionType.Relu,
                    bias=b1_sb[:, h:h + 1])
            # 4 node-interleaved matmuls: py partition p = SB row 4p + m,
            # so each partition's 4 Y rows are consecutive in yin
            py = psum_y.tile([P, 4, F_OUT], dt.float32, space="PSUM",
                             tag="py")
            for m in range(4):
                for h in range(2):
                    nc.tensor.matmul(
                        out=py[:, m, :],
                        lhsT=strided_cols(ht[:, h, :], m, 4, P),
                        rhs=w2_sb[:, h * F_OUT:(h + 1) * F_OUT],
                        start=(m == 0 and h == 0),
                        stop=(m == 3 and h == 1))
            yb = ypool.tile([P, 4 * F_OUT], dt.bfloat16, tag="yb")
            nc.vector.tensor_copy(yb[:], py[:, :, :])
            yv = yin[sb * SBW:(sb + 1) * SBW, :]
            dst = bass.AP(yv.tensor, yv.offset,
                          [[4 * F_OUT, P], [1, 4 * F_OUT]])
            nc.sync.dma_start(dst, yb[:])

        layer(featN[:], F_IN, fdt, 2, l1_out)

        if _os.environ.get("GCN_SKIP_L2"):
            psum_y_ctx.__exit__(None, None, None)
            psum_h2_ctx.__exit__(None, None, None)
            psum_ctx.__exit__(None, None, None)
            return

        if cfg.ncores > 1 and not _os.environ.get("TINY_AG"):
            nc.gpsimd.collective_compute(
                "AllGather", mybir.AluOpType.bypass, replica_groups=rg,
                ins=[yin.opt()], outs=[yall.opt()])
        else:
            nc.sync.dma_start(yall[0:cfg.bp, :], yin[:])

        # ------- layer 2 output -------
        def l2_out(sb, phs):
            ob = hpool.tile([P, SBW], dt.float32, tag="ob")
            nc.scalar.activation(
                ob[:], phs[0][:], mybir.ActivationFunctionType.Identity,
                bias=b2_sb[:, 0:1])
            # outT is [nsb, F_OUT, SBW]: one 2KB-per-partition write per SB
            nc.sync.dma_start(outT[sb], ob[:])

        layer(yall[:], F_OUT, dt.bfloat16, 1, l2_out)
        psum_y_ctx.__exit__(None, None, None)
        psum_h2_ctx.__exit__(None, None, None)
        psum_ctx.__exit__(None, None, None)


# --------------------------------------------------------------------------
# Top level
# --------------------------------------------------------------------------

def declare_io(nc, cfg):
    from concourse import mybir
    dt = mybir.dt

    def di(name, shape, d):
        return nc.dram_tensor(name, shape, d, kind="ExternalInput").ap()
    ins = {
        "featN": di("featN", [cfg.ntot, F_IN],
                    dt.float8e4 if FP8_FEAT else dt.bfloat16),
        "w1": di("w1", [2, P, F_HID], dt.bfloat16),
        "w2": di("w2", [2, P, F_OUT], dt.bfloat16),
        "b1": di("b1", [2, P, 1], dt.float32),
        "b2": di("b2", [P, 1], dt.float32),
        "iota": di("iota", [P, RB * SG], dt.bfloat16),
        "idxs": di("idxs", [P, cfg.totch * 8], dt.int16),
        "segid": di("segid", [P, cfg.totch], dt.bfloat16),
        "wgt": di("wgt", [P, cfg.totch], dt.bfloat16),
    }
    outs = {
        "outT": nc.dram_tensor("outT", [cfg.nsb, F_OUT, cfg.sbb * RB],
                               dt.float32, kind="ExternalOutput").ap(),
    }
    return ins, outs


def build_nc(cfg, meta, repeat=1):
    import sys
    sys.path.insert(0, "/opt/trn_rl_repo")
    from concourse import bacc, tile
    scratch = int(_os_mod.environ.get("GCN_DMA_SCRATCH", "16384"))
    nc = bacc.Bacc("TRN2", target_bir_lowering=False, debug=False,
                   num_devices=cfg.ncores, num_swdge_queues=NQ,
                   dynamic_dma_scratch_size=scratch)
    ins, outs = declare_io(nc, cfg)
    with tile.TileContext(nc) as tc:
        for _ in range(repeat):
            build_program(tc, cfg, meta, outs, ins)
    nc.compile()
    return nc


def kernel(feat, row, col, edge_weight, W1, b1, W2, b2):
    import sys
    sys.path.insert(0, "/opt/trn_rl_repo")
    feat = np.asarray(feat, dtype=np.float32)
    row = np.asarray(row, dtype=np.int32)
    col = np.asarray(col, dtype=np.int32)
    edge_weight = np.asarray(edge_weight, dtype=np.float32)
    W1 = np.asarray(W1, dtype=np.float32)
    b1 = np.asarray(b1, dtype=np.float32)
    W2 = np.asarray(W2, dtype=np.float32)
    b2 = np.asarray(b2, dtype=np.float32)

    cfg = full_cfg()
    assert feat.shape == (cfg.n, F_IN) and row.shape == (cfg.e,)

    in_maps, meta = preprocess(cfg, feat, row, col, edge_weight, W1, b1, W2, b2)
    nc = build_nc(cfg, meta)

    from concourse.bass_utils import run_bass_kernel_spmd
    res = run_bass_kernel_spmd(nc, in_maps, core_ids=list(range(cfg.ncores)))
    outs = [{"outT": r["outT"]} for r in res.results]
    return assemble(cfg, meta, outs)



# revision 17
# speedup vs baseline: 2.7271x; 1.3541x over previous
"""2-layer GCN (SpMM message passing) on 8 Trainium2 NeuronCores — v3.

Strategy (dest-row-sharded SpMM, fused layer-1):
  - Dest nodes are relabeled to (core, block, slot): 8 cores x BLOCKS blocks
    x 64 slots.  Per-core bin packing (snake by degree + repair) bounds each
    block's in-edge count by CPB*128.
  - Layer 1 is fused: since A@(X@W1) == (A@X)@W1, each core gathers raw
    fp8 feat rows per edge, aggregates A@X per super-block in PSUM, then
    applies W1 to the tiny [256, 512] aggregate (no replicated full-Z
    matmul phase, no Z DRAM round-trip).
  - Per core, edges are grouped by dest block and sorted by source col;
    the 128-edge chunks of a super-block (8 blocks) are interleaved by col
    band and packed into dma_gather calls of up to MAXG chunks whose
    cross-core col span fits the int16 window (32767 rows).
  - Aggregation: per chunk j, matmul lhsT=gathered[128 edges, feat-half],
    rhs=S cols (one-hot(slot)*w built on DVE per call) accumulating into
    the chunk's block slice of a [128, 512] PSUM bank per feat half.
  - Per super-block: agg -> bf16, H^T = relu(W1^T@agg + b1), Y = H @ W2
    per 128-row pair, then AllGather Y and layer 2 runs the same schedule
    on yall.
"""

import numpy as np

P = 128
RB = 64            # rows (slots) per block
F_IN = 256
F_HID = 256
F_OUT = 128
WIN = 32767        # dma_gather int16 index window
NQ = 4             # SWDGE queues for gathers (ucode max)
import os as _os_mod
FP8_FEAT = _os_mod.environ.get("GCN_FP8_FEAT", "1") == "1"
SG = 32            # chunks per S-matrix generation group (amortizes DVE ops)


class Cfg:
    def __init__(self, n_nodes, n_edges, ncores, cpb, maxg=48, sbb=8):
        assert n_nodes % ncores == 0
        self.n = n_nodes
        self.e = n_edges
        self.ncores = ncores
        self.npc = n_nodes // ncores
        nb = (self.npc + RB - 1) // RB
        if nb * RB - self.npc < 32:      # slack rows for bin packing
            nb += 1
        if nb % sbb:
            nb += sbb - nb % sbb         # whole super-blocks
        self.blocks = nb
        self.sbb = sbb                   # blocks per super-block
        self.nsb = nb // sbb
        self.cpb = cpb                   # chunks (of 128 edges) per block
        self.maxg = maxg                 # max chunks per dma_gather call
        self.totch = nb * cpb
        self.bp = nb * RB                # padded nodes per core
        self.ntot = ncores * self.bp


def full_cfg():
    # per-block capacity cpb*128=2048 vs ~2000 mean edges; bin packing keeps
    # every block under cap (assert-checked).
    import os
    maxg = int(os.environ.get("GCN_MAXG", "8"))  # >8 overflows the Q7 idx scratch and hangs
    return Cfg(100000, 3200000, 8, cpb=16, maxg=maxg, sbb=8)


# --------------------------------------------------------------------------
# Host-side preprocessing
# --------------------------------------------------------------------------

def _assign_nodes(cfg, deg_in):
    """node -> (core, block, slot), balancing per-core and per-block edges."""
    n, ncores, nb = cfg.n, cfg.ncores, cfg.blocks
    order = np.argsort(-deg_in, kind="stable")
    pos = np.arange(n)
    phase = pos % (2 * ncores)
    core_of_pos = np.where(phase < ncores, phase, 2 * ncores - 1 - phase)
    node_core = np.empty(n, dtype=np.int64)
    node_core[order] = core_of_pos

    node_block = np.empty(n, dtype=np.int64)
    node_slot = np.empty(n, dtype=np.int64)
    cap_edges = cfg.cpb * P
    for c in range(ncores):
        nodes = order[core_of_pos == c]          # degree-desc within core
        m = len(nodes)
        assert m == cfg.npc
        bpos = np.arange(m)
        ph = bpos % (2 * nb)
        blk = np.where(ph < nb, ph, 2 * nb - 1 - ph)
        cnt = np.bincount(blk, minlength=nb)
        esum = np.bincount(blk, weights=deg_in[nodes], minlength=nb)
        assert cnt.max() <= RB, f"block row overflow {cnt.max()}"
        if esum.max() > cap_edges:
            blk = blk.copy()
            for b in np.where(esum > cap_edges)[0]:
                members = np.where(blk == b)[0]
                members = members[np.argsort(deg_in[nodes[members]])]
                k = 0
                while esum[b] > cap_edges and k < len(members):
                    mv = members[k]
                    d = deg_in[nodes[mv]]
                    cands = np.where((esum + d <= cap_edges) & (cnt < RB))[0]
                    if len(cands) == 0:
                        raise RuntimeError("bin packing failed; raise cpb")
                    tgt = cands[np.argmin(esum[cands])]
                    blk[mv] = tgt
                    esum[b] -= d
                    esum[tgt] += d
                    cnt[b] -= 1
                    cnt[tgt] += 1
                    k += 1
            assert esum.max() <= cap_edges
        slot = np.zeros(m, dtype=np.int64)
        so = np.argsort(blk, kind="stable")
        sb = blk[so]
        start = np.r_[0, np.flatnonzero(np.diff(sb)) + 1]
        sizes = np.diff(np.r_[start, m])
        ranks = np.arange(m) - np.repeat(start, sizes)
        slot[so] = ranks
        node_block[nodes] = blk
        node_slot[nodes] = slot
    return node_core, node_block, node_slot


def preprocess(cfg, feat, row, col, edge_weight, W1, b1, W2, b2):
    from concourse import mybir
    bf16 = mybir.dt.np(mybir.dt.bfloat16)
    n = cfg.n
    ncores, nb, cpb = cfg.ncores, cfg.blocks, cfg.cpb
    deg_in = np.bincount(row, minlength=n)
    node_core, node_block, node_slot = _assign_nodes(cfg, deg_in)
    newid = node_core * cfg.bp + node_block * RB + node_slot

    # ---- per (core, block): edges sorted by col ----
    ec_new = newid[col]
    e_core = node_core[row]
    e_blk = node_block[row]
    e_slot = node_slot[row]
    gblk = e_core * nb + e_blk
    so = np.lexsort((ec_new, gblk))
    gblk_s = gblk[so]
    ec_s = ec_new[so]
    slot_s = e_slot[so]
    w_s = edge_weight[so]

    nblk_g = ncores * nb
    blk_cnt = np.bincount(gblk_s, minlength=nblk_g)
    cap_edges = cpb * P
    assert blk_cnt.max() <= cap_edges, f"{blk_cnt.max()} > {cap_edges}"
    blk_start = np.r_[0, np.cumsum(blk_cnt)[:-1]]
    rank_in_blk = np.arange(len(so)) - np.repeat(blk_start, blk_cnt)

    # chunk-level min/max col per (core, block, k) -> cross-core union span
    big = np.int64(1 << 60)
    e_chunk = rank_in_blk // P                      # k within block
    key = gblk_s * cpb + e_chunk                    # (core, block, k)
    cmin = np.full(nblk_g * cpb, big, dtype=np.int64)
    cmax = np.full(nblk_g * cpb, -1, dtype=np.int64)
    np.minimum.at(cmin, key, ec_s)
    np.maximum.at(cmax, key, ec_s)
    u_min = cmin.reshape(ncores, nb, cpb).min(axis=0)   # [nb, cpb]
    u_max = cmax.reshape(ncores, nb, cpb).max(axis=0)

    # ---- shared schedule: per super-block, order chunks by col band and
    # pack into calls with cross-core span <= WIN and <= maxg chunks ----
    chunk_order = []                                # (block, k) stream order
    call_start, call_len, call_base = [], [], []
    t = 0
    for s in range(cfg.nsb):
        items = []
        for b in range(s * cfg.sbb, (s + 1) * cfg.sbb):
            for k in range(cpb):
                mn = u_min[b, k]
                items.append((int(mn) if mn != big else 0,
                              int(u_max[b, k]), b, k))
        items.sort()
        cur0, curn, lo, hi = t, 0, big, -1
        for mn, mx, b, k in items:
            nlo = min(lo, mn) if mx >= 0 else lo
            nhi = max(hi, mx)
            span_ok = (nhi < 0) or (nlo == big) or (nhi - nlo <= WIN - 1)
            if curn > 0 and (curn >= cfg.maxg or not span_ok):
                call_start.append(cur0)
                call_len.append(curn)
                call_base.append(0 if lo == big else
                                 min(int(lo), max(cfg.ntot - WIN, 0)))
                cur0, curn, lo, hi = t, 0, big, -1
                nlo = mn if mx >= 0 else big
                nhi = mx
            lo, hi = nlo, nhi
            chunk_order.append((b, k))
            curn += 1
            t += 1
        call_start.append(cur0)
        call_len.append(curn)
        call_base.append(0 if lo == big else
                         min(int(lo), max(cfg.ntot - WIN, 0)))
    assert t == cfg.totch
    call_start = np.asarray(call_start)
    call_len = np.asarray(call_len)
    call_base = np.asarray(call_base, dtype=np.int64)
    call_width = np.minimum(WIN, cfg.ntot - call_base)

    # chunk meta in stream order
    chunk_block = np.asarray([b for b, _ in chunk_order])
    ck = np.asarray([k for _, k in chunk_order])
    stream_of = np.empty((nb, cpb), dtype=np.int64)
    stream_of[chunk_block, ck] = np.arange(cfg.totch)
    first = np.full(nb, cfg.totch, dtype=np.int64)
    last = np.full(nb, -1, dtype=np.int64)
    np.minimum.at(first, chunk_block, np.arange(cfg.totch))
    np.maximum.at(last, chunk_block, np.arange(cfg.totch))
    chunk_start = np.zeros(cfg.totch, dtype=bool)
    chunk_stop = np.zeros(cfg.totch, dtype=bool)
    chunk_start[first] = True
    chunk_stop[last] = True
    chunk_base = np.repeat(call_base, call_len)

    # ---- per-core padded planes in stream order ----
    tot = cfg.totch * P
    stream_pos = stream_of[gblk_s % nb, e_chunk] * P + rank_in_blk % P
    core_of_edge = gblk_s // nb
    colpad = np.zeros((ncores, tot), dtype=np.int64)
    slotpad = np.zeros((ncores, tot), dtype=np.int64)
    wpad = np.zeros((ncores, tot), dtype=np.float32)
    validpad = np.zeros((ncores, tot), dtype=bool)
    colpad[core_of_edge, stream_pos] = ec_s
    slotpad[core_of_edge, stream_pos] = slot_s
    wpad[core_of_edge, stream_pos] = w_s
    validpad[core_of_edge, stream_pos] = True

    cb_e = np.broadcast_to(np.repeat(chunk_base, P), (ncores, tot))
    colpad = np.where(validpad, colpad, cb_e)
    wpad = np.where(validpad, wpad, 0.0)
    wmax = int(np.where(validpad, colpad - cb_e, 0).max())
    assert wmax < WIN, f"window overflow {wmax}"
    assert (colpad >= cb_e).all()

    idx16 = (colpad - cb_e).astype(np.int16).reshape(ncores, cfg.totch, P)
    assert (idx16 >= 0).all()
    if _os_mod.environ.get("GCN_SEQ_IDX"):
        # diagnostic: sequential in-window indices (perfect HBM locality,
        # same descriptor count). Results are garbage; timing-only.
        cs = np.repeat(call_start, call_len)
        seq = ((np.arange(cfg.totch) - cs)[:, None] * P +
               np.arange(P)[None, :]).astype(np.int16)
        idx16 = np.broadcast_to(seq[None], (ncores, cfg.totch, P)).copy()
    idxp = idx16.reshape(ncores, cfg.totch, 8, 16)
    idxp = idxp.transpose(0, 3, 1, 2).reshape(ncores, 16, cfg.totch * 8)
    idx_plane = np.tile(idxp, (1, 8, 1))

    seg_plane = slotpad.reshape(ncores, cfg.totch, P).transpose(0, 2, 1)
    seg_plane = np.ascontiguousarray(seg_plane).astype(bf16)
    w_plane = wpad.reshape(ncores, cfg.totch, P).transpose(0, 2, 1)
    w_plane = np.ascontiguousarray(w_plane).astype(bf16)

    # iota plane [128, RB*sg] bf16: [p, s*sg + j] = s  (sg = S-gen group)
    iota_plane = np.repeat(np.arange(RB), SG).astype(bf16)
    iota_plane = np.tile(iota_plane, (P, 1))

    # full node-major feat [ntot, F_IN] (replicated to every core)
    fdt = mybir.dt.np(mybir.dt.float8e4) if FP8_FEAT else bf16
    feat_pad = np.zeros((cfg.ntot, F_IN), dtype=np.float32)
    feat_pad[newid] = feat
    featN = np.ascontiguousarray(feat_pad).astype(fdt)

    w1p = np.ascontiguousarray(W1.reshape(2, P, F_HID)).astype(bf16)
    w2p = np.ascontiguousarray(W2.reshape(2, P, F_OUT)).astype(bf16)
    b1p = np.ascontiguousarray(b1.reshape(2, P, 1)).astype(np.float32)
    b2p = np.ascontiguousarray(b2.reshape(P, 1)).astype(np.float32)

    in_maps = []
    for c in range(ncores):
        in_maps.append({
            "featN": featN,
            "w1": w1p, "w2": w2p, "b1": b1p, "b2": b2p,
            "iota": iota_plane,
            "idxs": np.ascontiguousarray(idx_plane[c]),
            "segid": seg_plane[c],
            "wgt": w_plane[c],
        })
    meta = {
        "call_start": call_start, "call_len": call_len,
        "base": call_base, "width": call_width,
        "chunk_block": chunk_block, "chunk_start": chunk_start,
        "chunk_stop": chunk_stop,
        "newid": newid, "node_core": node_core,
        "node_block": node_block, "node_slot": node_slot,
    }
    return in_maps, meta


def assemble(cfg, meta, outs):
    """outs: per core {'outT': [nsb, F_OUT, sbb*RB]} -> [n, F_OUT] f32."""
    res = np.empty((cfg.n, F_OUT), dtype=np.float32)
    nc_, nb_, ns_ = meta["node_core"], meta["node_block"], meta["node_slot"]
    for c in range(cfg.ncores):
        o = outs[c]["outT"]
        sel = np.where(nc_ == c)[0]
        b = nb_[sel]
        res[sel] = o[b // cfg.sbb, :, (b % cfg.sbb) * RB + ns_[sel]]
    return res


# --------------------------------------------------------------------------
# Device program
# --------------------------------------------------------------------------

def outer_bcast(ap, k):
    """Insert a step-0 dim of size k before the last dim."""
    from concourse import bass
    a = list(ap.ap)
    return bass.AP(ap.tensor, ap.offset, a[:-1] + [[0, k]] + a[-1:])


def strided_cols(ap, start, step, count):
    """Column view [p, start + step*j] of a contiguous [128, N] AP."""
    from concourse import bass
    a = list(ap.ap)
    return bass.AP(ap.tensor, ap.offset + start, [a[0], [step, count]])


def sgrp_view(tile_ap, maxg, nch, rb):
    """AP [p, s*maxg + j], s in [0, rb), j in [0, nch) of a [128, rb*maxg]
    contiguous tile (last dim packed -> DVE 2x mode)."""
    from concourse import bass
    a = list(tile_ap.ap)
    return bass.AP(tile_ap.tensor, tile_ap.offset, [a[0], [maxg, rb], [1, nch]])


def rows_view(dram_ap, r0, nrows, fel):
    """DRAM view of dram[r0:r0+nrows, :] ordered [p, j, f] with
    row = r0 + j*128 + p, matching an SBUF tile [128, nrows//128, fel]."""
    from concourse import bass
    a = [list(d) for d in dram_ap.ap]
    rs = a[0][0]                       # row stride (elements)
    assert a[1] == [1, fel], a
    return bass.AP(dram_ap.tensor, dram_ap.offset + r0 * rs,
                   [[rs, P], [rs * P, nrows // P], [1, fel]])


def build_program(tc, cfg, meta, outs, ins):
    import os as _os
    from concourse import mybir
    nc = tc.nc
    dt = mybir.dt
    base, width = meta["base"], meta["width"]
    call_start, call_len = meta["call_start"], meta["call_len"]
    chunk_block = meta["chunk_block"]
    chunk_start, chunk_stop = meta["chunk_start"], meta["chunk_stop"]
    featN, w1, w2 = ins["featN"], ins["w1"], ins["w2"]
    b1, b2 = ins["b1"], ins["b2"]
    iota, idxs, segid, wgt = ins["iota"], ins["idxs"], ins["segid"], ins["wgt"]
    outT = outs["outT"]
    SBB, MAXG = cfg.sbb, cfg.maxg
    SBW = SBB * RB                          # psum cols per super-block (512)
    SBCH = SBB * cfg.cpb                    # chunks per super-block
    rg = [list(range(cfg.ncores))]
    calls = len(call_start)

    with tc.tile_pool(name="const", bufs=1) as const, \
         tc.tile_pool(name="dram", bufs=1, space="DRAM") as dram, \
         tc.tile_pool(name="gpool", bufs=int(_os_mod.environ.get(
             "GCN_GBUFS", "12"))) as gpool, \
         tc.tile_pool(name="spool", bufs=4) as spool, \
         tc.tile_pool(name="hpool", bufs=3) as hpool, \
         tc.tile_pool(name="ypool", bufs=3) as ypool:
        # ------- constants -------
        idx_sb = const.tile([P, cfg.totch * 8], dt.int16)
        nc.sync.dma_start(idx_sb[:], idxs[:])
        seg_sb = const.tile([P, cfg.totch], dt.bfloat16)
        nc.sync.dma_start(seg_sb[:], segid[:])
        w_sb = const.tile([P, cfg.totch], dt.bfloat16)
        nc.sync.dma_start(w_sb[:], wgt[:])
        iota_sb = const.tile([P, RB * SG], dt.bfloat16)
        nc.sync.dma_start(iota_sb[:], iota[:])
        w1_sb = const.tile([P, 2 * F_HID], dt.bfloat16)
        nc.sync.dma_start(w1_sb[:, 0:F_HID], w1[0])
        nc.sync.dma_start(w1_sb[:, F_HID:2 * F_HID], w1[1])
        w2_sb = const.tile([P, 2 * F_OUT], dt.bfloat16)
        nc.sync.dma_start(w2_sb[:, 0:F_OUT], w2[0])
        nc.sync.dma_start(w2_sb[:, F_OUT:2 * F_OUT], w2[1])
        b1_sb = const.tile([P, 2], dt.float32)
        nc.sync.dma_start(b1_sb[:, 0:1], b1[0])
        nc.sync.dma_start(b1_sb[:, 1:2], b1[1])
        b2_sb = const.tile([P, 1], dt.float32)
        nc.sync.dma_start(b2_sb[:], b2[:])

        fdt = dt.float8e4 if FP8_FEAT else dt.bfloat16
        yin = dram.tile([cfg.bp, F_OUT], dt.bfloat16)
        yall = dram.tile([cfg.ntot, F_OUT], dt.bfloat16, addr_space="Shared")

        # ------- shared gather/aggregate layer -------
        psum_ctx = tc.tile_pool(name="psum_h", bufs=2, space="PSUM")
        psum_h = psum_ctx.__enter__()
        psum_h2_ctx = tc.tile_pool(name="psum_h2", bufs=1, space="PSUM")
        psum_h2 = psum_h2_ctx.__enter__()
        psum_y_ctx = tc.tile_pool(name="psum_y", bufs=2, space="PSUM")
        psum_y = psum_y_ctx.__enter__()

        def layer(src, felem, gdt, nhalves, out_cb):
            ph = {}                         # super-block -> psum tiles
            sgen = {}                       # S-gen group -> S tile

            def get_S(g):
                if g in sgen:
                    return sgen[g]
                c0g = g * SG
                eq = spool.tile([P, RB * SG], dt.bfloat16, tag="eq",
                                name="eq")
                S = spool.tile([P, RB * SG], dt.bfloat16, tag="S", name="S")
                nc.vector.tensor_tensor(
                    out=eq[:], in0=iota_sb[:],
                    in1=outer_bcast(seg_sb[:, c0g:c0g + SG], RB),
                    op=mybir.AluOpType.is_equal)
                nc.vector.tensor_tensor(
                    out=S[:], in0=eq[:],
                    in1=outer_bcast(w_sb[:, c0g:c0g + SG], RB),
                    op=mybir.AluOpType.mult)
                sgen[g] = S
                return S

            for tcall in range(calls):
                c0, nch = int(call_start[tcall]), int(call_len[tcall])
                gt = gpool.tile([P, MAXG, felem], gdt, tag="gt")
                win = src[int(base[tcall]):int(base[tcall] + width[tcall]), :]
                if not _os.environ.get("SKIP_GATHER"):
                    nc.gpsimd.dma_gather(
                        gt[:, :nch, :], win, idx_sb[:, c0 * 8:(c0 + nch) * 8],
                        nch * P, nch * P, felem, elem_step=felem,
                        queue_num=(tcall % NQ))
                else:
                    nc.vector.tensor_copy(gt[:, 0, 0:P], seg_sb[:, 0:P])
                if _os.environ.get("GCN_SKIP_MM"):
                    continue       # timing ablation: gathers only
                for j in range(nch):
                    c = c0 + j
                    b = int(chunk_block[c])
                    sb = b // SBB
                    S = get_S(c // SG)
                    if sb not in ph:
                        ph[sb] = [psum_h.tile([P, SBW], dt.float32,
                                              space="PSUM", tag=f"ph{h}",
                                              name=f"ph{h}")
                                  for h in range(nhalves)]
                    col0 = (b % SBB) * RB
                    # one psum group per bank per super-block (the start
                    # marks the whole 2KB zero region; slices lazily zero
                    # on first write)
                    for h in range(nhalves):
                        nc.tensor.matmul(
                            out=ph[sb][h][:, col0:col0 + RB],
                            lhsT=gt[:, j, h * P:(h + 1) * P],
                            rhs=strided_cols(S[:], c % SG, SG, RB),
                            start=(c % SBCH == 0),
                            stop=(c % SBCH == SBCH - 1))
                    if c == (sb + 1) * SBCH - 1:
                        out_cb(sb, ph.pop(sb))

        # ------- layer 1 output: agg -> W1 -> relu + bias -> Y = H @ W2 ----
        def l1_out(sb, phs):
            from concourse import bass
            # phs: feat-in-major aggregate [2][128 fin, 512 slots] in PSUM.
            aggT = hpool.tile([P, 2, SBW], dt.bfloat16, tag="aggT")
            for h in range(2):
                nc.scalar.activation(
                    aggT[:, h, :], phs[h][:],
                    mybir.ActivationFunctionType.Copy)
            # H^T[hid, slot] = W1^T @ agg: lhsT = W1[fin, hid] per 128-half
            ph2 = [psum_h2.tile([P, SBW], dt.float32, space="PSUM",
                                tag=f"ph2{h}", name=f"ph2{h}")
                   for h in range(2)]
            for hh in range(2):
                for fh in range(2):
                    nc.tensor.matmul(
                        out=ph2[hh][:],
                        lhsT=w1_sb[:, fh * F_HID + hh * P:
                                   fh * F_HID + hh * P + P],
                        rhs=aggT[:, fh, :],
                        start=(fh == 0), stop=(fh == 1))
            ht = hpool.tile([P, 2, SBW], dt.bfloat16, tag="ht")
            for h in range(2):
                nc.scalar.activation(
                    ht[:, h, :], ph2[h][:],
                    mybir.ActivationFunctionType.Relu,
                    bias=b1_sb[:, h:h + 1])
            # 4 node-interleaved matmuls: py partition p = SB row 4p + m,
            # so each partition's 4 Y rows are consecutive in yin
            py = psum_y.tile([P, 4, F_OUT], dt.float32, space="PSUM",
                             tag="py")
            for m in range(4):
                for h in range(2):
                    nc.tensor.matmul(
                        out=py[:, m, :],
                        lhsT=strided_cols(ht[:, h, :], m, 4, P),
                        rhs=w2_sb[:, h * F_OUT:(h + 1) * F_OUT],
                        start=(m == 0 and h == 0),
                        stop=(m == 3 and h == 1))
            yb = ypool.tile([P, 4 * F_OUT], dt.bfloat16, tag="yb")
            nc.vector.tensor_copy(yb[:], py[:, :, :])
            yv = yin[sb * SBW:(sb + 1) * SBW, :]
            dst = bass.AP(yv.tensor, yv.offset,
                          [[4 * F_OUT, P], [1, 4 * F_OUT]])
            nc.sync.dma_start(dst, yb[:])

        layer(featN[:], F_IN, fdt, 2, l1_out)

        if _os.environ.get("GCN_SKIP_L2"):
            psum_y_ctx.__exit__(None, None, None)
            psum_h2_ctx.__exit__(None, None, None)
            psum_ctx.__exit__(None, None, None)
            return

        if cfg.ncores > 1 and not _os.environ.get("TINY_AG"):
            nc.gpsimd.collective_compute(
                "AllGather", mybir.AluOpType.bypass, replica_groups=rg,
                ins=[yin.opt()], outs=[yall.opt()])
        else:
            nc.sync.dma_start(yall[0:cfg.bp, :], yin[:])

        # ------- layer 2 output -------
        def l2_out(sb, phs):
            ob = hpool.tile([P, SBW], dt.float32, tag="ob")
            nc.scalar.activation(
                ob[:], phs[0][:], mybir.ActivationFunctionType.Identity,
                bias=b2_sb[:, 0:1])
            # outT is [nsb, F_OUT, SBW]: one 2KB-per-partition write per SB
            nc.sync.dma_start(outT[sb], ob[:])

        layer(yall[:], F_OUT, dt.bfloat16, 1, l2_out)
        psum_y_ctx.__exit__(None, None, None)
        psum_h2_ctx.__exit__(None, None, None)
        psum_ctx.__exit__(None, None, None)


# --------------------------------------------------------------------------
# Top level
# --------------------------------------------------------------------------

def declare_io(nc, cfg):
    from concourse import mybir
    dt = mybir.dt

    def di(name, shape, d):
        return nc.dram_tensor(name, shape, d, kind="ExternalInput").ap()
    ins = {
        "featN": di("featN", [cfg.ntot, F_IN],
                    dt.float8e4 if FP8_FEAT else dt.bfloat16),
        "w1": di("w1", [2, P, F_HID], dt.bfloat16),
        "w2": di("w2", [2, P, F_OUT], dt.bfloat16),
        "b1": di("b1", [2, P, 1], dt.float32),
        "b2": di("b2", [P, 1], dt.float32),
        "iota": di("iota", [P, RB * SG], dt.bfloat16),
        "idxs": di("idxs", [P, cfg.totch * 8], dt.int16),
        "segid": di("segid", [P, cfg.totch], dt.bfloat16),
        "wgt": di("wgt", [P, cfg.totch], dt.bfloat16),
    }
    outs = {
        "outT": nc.dram_tensor("outT", [cfg.nsb, F_OUT, cfg.sbb * RB],
                               dt.float32, kind="ExternalOutput").ap(),
    }
    return ins, outs


def build_nc(cfg, meta, repeat=1):
    import sys
    sys.path.insert(0, "/opt/trn_rl_repo")
    from concourse import bacc, tile
    scratch = int(_os_mod.environ.get("GCN_DMA_SCRATCH", "16384"))
    nc = bacc.Bacc("TRN2", target_bir_lowering=False, debug=False,
                   num_devices=cfg.ncores, num_swdge_queues=NQ,
                   dynamic_dma_scratch_size=scratch)
    ins, outs = declare_io(nc, cfg)
    with tile.TileContext(nc) as tc:
        for _ in range(repeat):
            build_program(tc, cfg, meta, outs, ins)
    nc.compile()
    return nc


def kernel(feat, row, col, edge_weight, W1, b1, W2, b2):
    import sys
    sys.path.insert(0, "/opt/trn_rl_repo")
    feat = np.asarray(feat, dtype=np.float32)
    row = np.asarray(row, dtype=np.int32)
    col = np.asarray(col, dtype=np.int32)
    edge_weight = np.asarray(edge_weight, dtype=np.float32)
    W1 = np.asarray(W1, dtype=np.float32)
    b1 = np.asarray(b1, dtype=np.float32)
    W2 = np.asarray(W2, dtype=np.float32)
    b2 = np.asarray(b2, dtype=np.float32)

    cfg = full_cfg()
    assert feat.shape == (cfg.n, F_IN) and row.shape == (cfg.e,)

    in_maps, meta = preprocess(cfg, feat, row, col, edge_weight, W1, b1, W2, b2)
    nc = build_nc(cfg, meta)

    from concourse.bass_utils import run_bass_kernel_spmd
    res = run_bass_kernel_spmd(nc, in_maps, core_ids=list(range(cfg.ncores)))
    outs = [{"outT": r["outT"]} for r in res.results]
    return assemble(cfg, meta, outs)

